# revision 8
# baseline (speedup 1.0000x reference)
"""Trainium2 Bass kernel for nn_PixelMultiheadAttention.

Per-pixel multihead attention: N=262144 independent pixels, LQ=1 query,
LK=3 keys, D=128, H=8 heads (head_dim 16). Pure data-parallel over 8
NeuronCores (N/8 = 32768 pixels per core).

Dataflow per 512-pixel supertile (feat-in-partitions layout):
  DMA natural tiles -> PE transpose (f32r, host-rounded inputs) ->
  ACT/DVE copy PSUM->SBUF -> f32r projection matmuls ->
  t_j = (q+bq) * k_j on DVE -> scores via 0/1 segment-sum matmuls
  (bk folded in via a phi-matmul, bq*bk via the exp bias) ->
  softmax without division: exp -> segment-sum matmul -> ln -> exp(-x) ->
  w = e * r -> head-broadcast expansion matmul -> u_j = W_j * v_j on DVE ->
  PSUM-accumulated out-projection (bv folded into the output bias) ->
  PE transpose back -> contiguous DMA store.
"""
import os
import numpy as np
from contextlib import ExitStack

import concourse.bass as bass
import concourse.tile as tile
from concourse import bacc, mybir
from concourse.bass_utils import run_bass_kernel_spmd

F32 = mybir.dt.float32
F32R = mybir.dt.float32r

N_CORES = 8
N = 262144
D = 128
H = 8
HD = 16
LK = 3
NP = N // N_CORES          # pixels per core = 32768
P = 512                    # pixels per supertile
NB = P // 128              # 128-pixel blocks per supertile = 4
NS = NP // P               # supertiles per core = 64
SCALE = 1.0 / np.sqrt(HD)  # 0.25

TRACE = bool(int(os.environ.get("KERNEL_TRACE", "0")))

_cache = {}


def _round_f32r(x: np.ndarray) -> np.ndarray:
    """Round fp32 to the f32r grid (drop low 12 mantissa bits, round-to-nearest)."""
    b = np.ascontiguousarray(x, dtype=np.float32).view(np.uint32)
    b = (b + 0x800) & np.uint32(0xFFFFF000)
    return b.view(np.float32)


def build(ns: int = NS):
    nc = bacc.Bacc("TRN2", target_bir_lowering=False, debug=False,
                   num_devices=N_CORES)

    npx = ns * P
    # Inputs laid out for supertile DMA: (s, b, p, [j,] d)
    Q = nc.dram_tensor("Q", [ns, NB, 128, D], F32R, kind="ExternalInput").ap()
    K = nc.dram_tensor("K", [ns, NB, 128, LK, D], F32R, kind="ExternalInput").ap()
    V = nc.dram_tensor("V", [ns, NB, 128, LK, D], F32R, kind="ExternalInput").ap()
    Y = nc.dram_tensor("Y", [ns, NB, 128, D], F32R, kind="ExternalOutput").ap()

    # Constant operands (all pre-transposed / packed on host)
    wqT = nc.dram_tensor("wqT", [D, D], F32R, kind="ExternalInput").ap()
    wkT = nc.dram_tensor("wkT", [D, D], F32R, kind="ExternalInput").ap()
    wvT = nc.dram_tensor("wvT", [D, D], F32R, kind="ExternalInput").ap()
    woT = nc.dram_tensor("woT", [D, D], F32R, kind="ExternalInput").ap()
    s8x = [nc.dram_tensor(f"s8x{j}", [D, 72], F32R, kind="ExternalInput").ap()
           for j in range(LK)]
    sphi = nc.dram_tensor("sphi", [D, 72], F32R, kind="ExternalInput").ap()
    t72 = nc.dram_tensor("t72", [72, 72], F32R, kind="ExternalInput").ap()
    s72x = [nc.dram_tensor(f"s72x{j}", [72, D], F32R, kind="ExternalInput").ap()
            for j in range(LK)]
    ident = nc.dram_tensor("ident", [128, 128], F32R, kind="ExternalInput").ap()
    bq = nc.dram_tensor("bq", [D, 1], F32, kind="ExternalInput").ap()
    c72 = nc.dram_tensor("c72", [72, 1], F32, kind="ExternalInput").ap()
    bo = nc.dram_tensor("bo", [D, 1], F32, kind="ExternalInput").ap()

    with tile.TileContext(nc) as tc, ExitStack() as ctx:
        cpool = ctx.enter_context(tc.tile_pool(name="consts", bufs=1))
        sb = ctx.enter_context(tc.tile_pool(name="sb", bufs=2))
        trp = ctx.enter_context(tc.tile_pool(name="trp", bufs=2, space="PSUM"))
        kvp = ctx.enter_context(tc.tile_pool(name="kvp", bufs=2, space="PSUM"))
        qp = ctx.enter_context(tc.tile_pool(name="qp", bufs=1, space="PSUM"))
        scp = ctx.enter_context(tc.tile_pool(name="scp", bufs=1, space="PSUM"))
        wp = ctx.enter_context(tc.tile_pool(name="wp", bufs=2, space="PSUM"))

        def const(ap_dram, shape, dtype, tag):
            t = cpool.tile(shape, dtype, tag=tag, name=f"c_{tag}")
            nc.sync.dma_start(t[:], ap_dram)
            return t

        wq_t = const(wqT, [D, D], F32R, "wq")
        wk_t = const(wkT, [D, D], F32R, "wk")
        wv_t = const(wvT, [D, D], F32R, "wv")
        wo_t = const(woT, [D, D], F32R, "wo")
        s8_t = [const(s8x[j], [D, 72], F32R, f"s8x{j}") for j in range(LK)]
        sphi_t = const(sphi, [D, 72], F32R, "sphi")
        t72_t = const(t72, [72, 72], F32R, "t72")
        s72_t = [const(s72x[j], [72, D], F32R, f"s72x{j}") for j in range(LK)]
        id_t = const(ident, [128, 128], F32R, "id")
        bq_t = const(bq, [D, 1], F32, "bq")
        c72_t = const(c72, [72, 1], F32, "c72")
        bo_t = const(bo, [D, 1], F32, "bo")

        for s in range(ns):
            # ---- load natural tiles --------------------------------------
            xq_nat = sb.tile([128, P], F32R, tag="xq_nat")
            nc.sync.dma_start(xq_nat[:], Q[s].transpose([1, 0, 2]))
            xk_nat = [sb.tile([128, P], F32R, tag=f"xk_nat{j}", name=f"xk_nat{j}_{s}") for j in range(LK)]
            xv_nat = [sb.tile([128, P], F32R, tag=f"xv_nat{j}", name=f"xv_nat{j}_{s}") for j in range(LK)]
            for j in range(LK):
                nc.sync.dma_start(xk_nat[j][:], K[s, :, :, j, :].transpose([1, 0, 2]))
                nc.sync.dma_start(xv_nat[j][:], V[s, :, :, j, :].transpose([1, 0, 2]))

            # ---- transpose to feat-parts + copy to SBUF ------------------
            def transpose_to_sbuf(nat, tag, eng):
                ps = trp.tile([128, P], F32R, tag="tr", name=f"trps_{s}_{tag}")
                for b in range(NB):
                    nc.tensor.transpose(ps[:, b * 128:(b + 1) * 128],
                                        nat[:, b * 128:(b + 1) * 128], id_t[:])
                out = sb.tile([128, P], F32R, tag=tag, name=f"xt_{s}_{tag}")
                if eng == "act":
                    nc.scalar.copy(out[:], ps[:])
                else:
                    nc.vector.tensor_copy(out[:], ps[:])
                return out

            xtq = transpose_to_sbuf(xq_nat[:], "xtq", "act")
            xtk = [transpose_to_sbuf(xk_nat[j][:], f"xtk{j}", "act") for j in range(LK)]
            xtv = [transpose_to_sbuf(xv_nat[j][:], f"xtv{j}", "act") for j in range(LK)]

            # ---- projections (f32r matmuls) ------------------------------
            q_ps = qp.tile([128, P], F32, tag="q")
            nc.tensor.matmul(q_ps[:], wq_t[:], xtq[:], start=True, stop=True)
            q_sb = sb.tile([128, P], F32, tag="q_sb")
            nc.scalar.activation(q_sb[:], q_ps[:],
                                 mybir.ActivationFunctionType.Identity,
                                 bias=bq_t[:], scale=1.0)

            scores = scp.tile([128, P], F32, tag="scores")
            t_tiles = []
            for j in range(LK):
                k_ps = kvp.tile([128, P], F32, tag="kv")
                nc.tensor.matmul(k_ps[:], wk_t[:], xtk[j][:], start=True, stop=True)
                t_j = sb.tile([128, P], F32R, tag=f"t{j}", name=f"t{j}_{s}")
                nc.vector.tensor_mul(t_j[:], q_sb[:], k_ps[:])
                t_tiles.append(t_j)
                # scores_j lands in rows 32j..32j+7 via the masked stationary
                nc.tensor.matmul(scores[0:72, :], s8_t[j][:], t_j[:],
                                 start=(j == 0), stop=False)
            # phi: (q_raw . bk) per head, all three j groups at once
            nc.tensor.matmul(scores[0:72, :], sphi_t[:], xtq[:],
                             start=False, stop=True)

            v_sb = []
            for j in range(LK):
                v_ps = kvp.tile([128, P], F32, tag="kv")
                nc.tensor.matmul(v_ps[:], wv_t[:], xtv[j][:], start=True, stop=True)
                vs = sb.tile([128, P], F32, tag=f"v_sb{j}", name=f"v_sb{j}_{s}")
                nc.vector.tensor_copy(vs[:], v_ps[:])
                v_sb.append(vs)

            # ---- softmax (no division) -----------------------------------
            e72 = sb.tile([72, P], F32R, tag="e72")
            nc.scalar.activation(e72[:], scores[0:72, :],
                                 mybir.ActivationFunctionType.Exp,
                                 bias=c72_t[:], scale=SCALE)
            # esum replicated into rows 0..71 of the scores bank
            nc.tensor.matmul(scores[0:72, :], t72_t[:], e72[:], start=True, stop=True)
            ln72 = sb.tile([72, P], F32, tag="ln72")
            nc.scalar.activation(ln72[:], scores[0:72, :],
                                 mybir.ActivationFunctionType.Ln)
            r72 = sb.tile([72, P], F32R, tag="r72")
            nc.scalar.activation(r72[:], ln72[:],
                                 mybir.ActivationFunctionType.Exp, scale=-1.0)
            w72 = sb.tile([72, P], F32R, tag="w72")
            nc.vector.tensor_mul(w72[:], e72[:], r72[:])

            # ---- expand weights, weight V, out-projection ----------------
            # reuses the q slot (q_ps is dead after the q_sb copy)
            y_ps = qp.tile([128, P], F32, tag="q")
            for j in range(LK):
                w_ps = wp.tile([128, P], F32, tag="W", name=f"w_ps{j}_{s}")
                nc.tensor.matmul(w_ps[:], s72_t[j][:], w72[:],
                                 start=True, stop=True)
                u_j = sb.tile([128, P], F32R, tag=f"u{j}", name=f"u{j}_{s}")
                nc.vector.tensor_mul(u_j[:], v_sb[j][:], w_ps[:])
                nc.tensor.matmul(y_ps[:], wo_t[:], u_j[:],
                                 start=(j == 0), stop=(j == LK - 1))

            y_sb = sb.tile([128, P], F32R, tag="y_sb")
            nc.scalar.activation(y_sb[:], y_ps[:],
                                 mybir.ActivationFunctionType.Identity,
                                 bias=bo_t[:], scale=1.0)

            # ---- transpose back + store ----------------------------------
            yt_ps = trp.tile([128, P], F32R, tag="tr", name=f"trps_{s}_yt")
            for b in range(NB):
                nc.tensor.transpose(yt_ps[:, b * 128:(b + 1) * 128],
                                    y_sb[:, b * 128:(b + 1) * 128], id_t[:])
            yt_sb = sb.tile([128, P], F32R, tag="yt_sb")
            nc.vector.tensor_copy(yt_sb[:], yt_ps[:])
            nc.sync.dma_start(Y[s].transpose([1, 0, 2]), yt_sb[:])

    nc.compile()
    return nc


def _prep_consts(in_proj_w, in_proj_b, out_w, out_b):
    Wq, Wk, Wv = in_proj_w[:D], in_proj_w[D:2 * D], in_proj_w[2 * D:]
    bq_, bk_, bv_ = in_proj_b[:D], in_proj_b[D:2 * D], in_proj_b[2 * D:]

    hh = np.arange(D) // HD  # head of each feature index
    s8 = np.zeros((D, H), np.float32)
    s8[np.arange(D), hh] = 1.0
    # per-j masked stationaries: s8x[j][d, 32j+h] = delta(head(d), h)
    s8x = []
    s72x = []
    for j in range(LK):
        m = np.zeros((D, 72), np.float32)
        m[:, 32 * j:32 * j + H] = s8
        s8x.append(m)
        m2 = np.zeros((72, D), np.float32)
        m2[32 * j:32 * j + H, :] = s8.T
        s72x.append(m2)

    # sphi[d, 32j+h] = sum_{i in head h} bk[16h+i] * Wq[16h+i, d]
    phi_cols = np.zeros((D, H), np.float32)
    for h in range(H):
        phi_cols[:, h] = Wq[16 * h:16 * h + 16].T @ bk_[16 * h:16 * h + 16]
    sphi = np.zeros((D, 72), np.float32)
    for j in range(LK):
        sphi[:, 32 * j:32 * j + H] = phi_cols

    # t72[32j+h, 32j'+h'] = delta(h, h') for all j, j'
    t72 = np.zeros((72, 72), np.float32)
    for j in range(LK):
        for jp in range(LK):
            t72[32 * j:32 * j + H, 32 * jp:32 * jp + H] = np.eye(H, dtype=np.float32)
    # identity on the filler rows so esum there is 1.0 (not 0 -> ln(0) -> NaN)
    for r in range(72):
        if (r % 32) >= H:
            t72[r, r] = 1.0

    c_h = np.array([bq_[16 * h:16 * h + 16] @ bk_[16 * h:16 * h + 16]
                    for h in range(H)], np.float32)
    c72 = np.zeros((72, 1), np.float32)
    for j in range(LK):
        c72[32 * j:32 * j + H, 0] = SCALE * c_h

    bo_full = (out_w @ bv_ + out_b).astype(np.float32)

    return {
        "wqT": _round_f32r(Wq.T), "wkT": _round_f32r(Wk.T),
        "wvT": _round_f32r(Wv.T), "woT": _round_f32r(out_w.T),
        "s8x0": s8x[0], "s8x1": s8x[1], "s8x2": s8x[2],
        "s72x0": s72x[0], "s72x1": s72x[1], "s72x2": s72x[2],
        "sphi": _round_f32r(sphi), "t72": t72,
        "ident": np.eye(128, dtype=np.float32),
        "bq": bq_.reshape(D, 1).astype(np.float32),
        "c72": c72, "bo": bo_full.reshape(D, 1),
    }


def kernel(Q, K, V, in_proj_w, in_proj_b, out_w, out_b):
    Q = np.asarray(Q, np.float32).reshape(N, D)
    K = np.asarray(K, np.float32).reshape(N, LK * D)
    V = np.asarray(V, np.float32).reshape(N, LK * D)
    consts = _prep_consts(np.asarray(in_proj_w, np.float32),
                          np.asarray(in_proj_b, np.float32),
                          np.asarray(out_w, np.float32),
                          np.asarray(out_b, np.float32))

    if "nc" not in _cache:
        _cache["nc"] = build()
    nc = _cache["nc"]

    Qr = _round_f32r(Q).reshape(N_CORES, NS, NB, 128, D)
    Kr = _round_f32r(K).reshape(N_CORES, NS, NB, 128, LK, D)
    Vr = _round_f32r(V).reshape(N_CORES, NS, NB, 128, LK, D)

    in_maps = []
    for c in range(N_CORES):
        m = {"Q": Qr[c], "K": Kr[c], "V": Vr[c]}
        m.update(consts)
        in_maps.append(m)

    res = run_bass_kernel_spmd(nc, in_maps, core_ids=list(range(N_CORES)),
                               trace=TRACE)
    _cache["last_result"] = res
    y = np.concatenate([r["Y"].reshape(NP, D) for r in res.results], axis=0)
    return y.reshape(N, 1, D)
